# revision 1
# baseline (speedup 1.0000x reference)
"""GAT message-passing kernel for 8 TRN2 NeuronCores (Bass/Tile).

v4 strategy (dst-sharded, segmented dma_gather, no collectives):
  - Each core owns a contiguous range of destination nodes; the host routes
    each edge to the core owning its destination (edge_index[1]).
  - The core's dsts are split into 8 segments so that each segment's
    referenced node set (its dsts + the sources of their edges) fits in
    int16 index space (<= 32767 rows).  Per segment the host packs a
    compacted 256B-row table XP[row] = [1.0 | x[n] bf16 x64 | 0 pad] —
    layout only, no math.
  - Edges grouped per destination into blocks of S = deg+1 slots (slot 0 =
    the destination's own row), exact-degree buckets, 128-slot tiles — the
    v1 bucket/mask machinery.  Slot indices are emitted as int16 in
    dma_gather's wrapped-16 layout.
  - Device: one dma_gather per compute chunk (~hundreds-thousands of rows,
    ~2.7ns/row with queue_num rotated over 4 SWDGE queues = 4 Q7 pairs
    generating descriptors in parallel).  Scores are computed post-gather:
    PE-transpose of the gathered x block, then a [128,4] matmul against
    [w_j|w_i] per tile pair.  exp(leakyrelu(s_i + s_j)) per slot; the
    aggregation matmul is transposed — U2[cols, 65] = exsel^T-contracted
    with G — so columns land on partitions and the softmax division is
    3 cheap partition-parallel ops.  Output written as [cols, 65].
  - Host inverts the block permutation and assembles the full output.
"""
import numpy as np

N_NODES = 100000
HIDDEN = 64
N_CORES = 8
NSEG = 8                 # dst segments per core (int16 index headroom)
LEAKY = 0.01
P = 128
ROW = 128                # bf16 elements per table row (256B, dma_gather)
NQUEUES = 4
CLAMP = 30.0


def _build_layout(edge_src, edge_dst_local, nodes_per_core):
    """Per-core, per-segment compact tables + tile/bucket structure.

    Returns per-core host arrays plus the shared program structure:
      program = list of (bucket_key d, n_tiles, m) per (segment, bucket),
      with segment table base rows and per-call index layout shared.
    """
    ncores = len(edge_src)
    npseg = nodes_per_core // NSEG

    # per (core, seg): sorted edges, per-dst slices
    seg_info = []  # [core][seg] -> (deg array, srcs per dst dict)
    for c in range(ncores):
        src, dstl = edge_src[c], edge_dst_local[c]
        order = np.argsort(dstl, kind="stable")
        src, dstl = src[order], dstl[order]
        deg = np.bincount(dstl, minlength=nodes_per_core)
        starts = np.concatenate([[0], np.cumsum(deg)])
        per_seg = []
        for s in range(NSEG):
            lo, hi = s * npseg, (s + 1) * npseg
            per = {}
            for n in range(lo, hi):
                d = int(deg[n])
                if d == 0:
                    continue
                per.setdefault(d, []).append(
                    (n, src[starts[n]:starts[n + 1]]))
            per_seg.append(per)
        seg_info.append(per_seg)

    # shared bucket structure: per segment, union of degrees across cores;
    # n_tiles = max over cores
    program = []  # (seg, d, n_tiles, m)
    for s in range(NSEG):
        all_d = sorted({d for c in range(ncores)
                        for d in seg_info[c][s].keys()})
        for d in all_d:
            if d <= 0 or d > 126:
                raise ValueError(f"unsupported degree {d}")
            S = d + 1
            m = min(P // S, 32)
            maxb = max(len(seg_info[c][s].get(d, [])) for c in range(ncores))
            n_tiles = (maxb + m - 1) // m
            program.append((s, d, n_tiles, m))

    NBT = 7
    total_tiles = sum(p[2] for p in program)
    total_idx = total_tiles * P
    total_cols = sum(p[2] * p[3] for p in program)

    # per-core: compact maps, tables assembled later; J int16 stream in
    # dma_gather order (idx i of a call -> slot (i%128, tile i//128)),
    # i.e. per tile 128 consecutive idxs.
    Js = []       # [core] -> int16 [total_idx]
    colmaps = []  # [core] -> int32 [total_cols] (local dst id or -1)
    seg_nodes = []  # [core][seg] -> int64 array of global node ids
    for c in range(ncores):
        J = np.zeros(total_idx, dtype=np.int16)
        colmap = np.full(total_cols, -1, dtype=np.int32)
        pernodes = []
        i0 = 0
        c0 = 0
        base_global = c * nodes_per_core
        for s in range(NSEG):
            # compact node set for this (core, seg)
            srcs_all = [srcs for d, lst in seg_info[c][s].items()
                        for (_, srcs) in lst]
            dsts_all = np.arange(s * npseg, (s + 1) * npseg) + base_global
            allref = np.concatenate(
                [np.concatenate(srcs_all) if srcs_all else
                 np.empty(0, dtype=np.int64), dsts_all])
            nodes = np.unique(allref)
            assert len(nodes) <= 32767, len(nodes)
            pernodes.append(nodes)
            lut = {}
            for j, n in enumerate(nodes):
                lut[int(n)] = j
            for (s2, d, n_tiles, m) in program:
                if s2 != s:
                    continue
                S = d + 1
                lst = seg_info[c][s].get(d, [])
                # column order mirrors the device chunking: per chunk of
                # nb tiles, col = chunk_c0 + b*nb + k
                chunk_c0 = {}
                cc = c0
                t = 0
                while t < n_tiles:
                    nb = min(NBT, n_tiles - t)
                    chunk_c0[t] = (cc, nb)
                    cc += m * nb
                    t += nb
                for bi, (n, srcs) in enumerate(lst):
                    t, b = bi // m, bi % m
                    base = i0 + t * P + b * S
                    J[base] = lut[int(n) + base_global]
                    for e, sv in enumerate(srcs):
                        J[base + 1 + e] = lut[int(sv)]
                    tc = (t // NBT) * NBT
                    ccc, nb = chunk_c0[tc]
                    colmap[ccc + b * nb + (t - tc)] = n
                i0 += n_tiles * P
                c0 += n_tiles * m
        Js.append(J)
        colmaps.append(colmap)
        seg_nodes.append(pernodes)
    return program, total_tiles, total_cols, Js, colmaps, seg_nodes


def _build_masks(program):
    """Per-bucket block masks (BM excludes slot0) and slot0 selectors."""
    import ml_dtypes

    keys = sorted({(d, m) for (_, d, _, m) in program})
    bm, sm, key_idx = [], [], {}
    for ki, (d, m) in enumerate(keys):
        S = d + 1
        B = np.zeros((P, m), dtype=np.float32)
        SEL = np.zeros((P, P), dtype=np.float32)
        for p in range(m * S):
            if p % S != 0:
                B[p, p // S] = 1.0
            SEL[(p // S) * S, p] = 1.0
        bm.append(B)
        sm.append(SEL)
        key_idx[(d, m)] = ki
    bmc = np.concatenate(bm, 1).astype(ml_dtypes.bfloat16)
    smc = np.concatenate(sm, 1).astype(ml_dtypes.bfloat16)
    bm_off = np.cumsum([0] + [b.shape[1] for b in bm])
    return bmc, smc, bm_off, key_idx


def _build_program(program, total_tiles, total_cols, seg_rows, n_bm_cols,
                   nbuckets):
    import concourse.bass as bass
    import concourse.tile as tile
    from concourse import bacc, mybir, library_config
    from concourse.mybir import ActivationFunctionType as AFT

    total_idx = total_tiles * P
    seg_base = np.cumsum([0] + list(seg_rows))

    nc = bacc.Bacc("TRN2", target_bir_lowering=False,
                   num_swdge_queues=NQUEUES,
                   dynamic_dma_scratch_size=65536)
    XP = nc.dram_tensor("XP", [int(seg_base[-1]), ROW], mybir.dt.bfloat16,
                        kind="ExternalInput")
    IDX = nc.dram_tensor("IDX", [P, total_idx // 16], mybir.dt.int16,
                         kind="ExternalInput")
    W2R = nc.dram_tensor("W2R", [P, 2 * HIDDEN], mybir.dt.bfloat16,
                         kind="ExternalInput")
    BM = nc.dram_tensor("BM", [P, n_bm_cols], mybir.dt.bfloat16,
                        kind="ExternalInput")
    SM = nc.dram_tensor("SM", [P, P * nbuckets], mybir.dt.bfloat16,
                        kind="ExternalInput")
    OUT = nc.dram_tensor("OUT", [total_cols, HIDDEN + 1], mybir.dt.float32,
                         kind="ExternalOutput")

    with tile.TileContext(nc) as tc:
        with (
            tc.tile_pool(name="msk", bufs=1) as mskp,
            tc.tile_pool(name="g", bufs=8) as gp,
            tc.tile_pool(name="xt", bufs=3) as xtp,
            tc.tile_pool(name="sc", bufs=4) as scp,
            tc.tile_pool(name="fl", bufs=4) as flp,
            tc.tile_pool(name="psT", bufs=2, space="PSUM") as psT,
            tc.tile_pool(name="psS", bufs=2, space="PSUM") as psS,
            tc.tile_pool(name="psU", bufs=2, space="PSUM") as psU,
        ):
            nc.gpsimd.load_library(library_config.mlp)
            bmall = mskp.tile([P, n_bm_cols], mybir.dt.bfloat16)
            nc.sync.dma_start(bmall[:], BM[:])
            small = mskp.tile([P, P * nbuckets], mybir.dt.bfloat16)
            nc.sync.dma_start(small[:], SM[:])
            w2r = mskp.tile([P, 2 * HIDDEN], mybir.dt.bfloat16)
            nc.sync.dma_start(w2r[:], W2R[:])
            jtall = mskp.tile([P, total_idx // 16], mybir.dt.int16)
            nc.sync.dma_start(jtall[:], IDX[:])

            i0 = 0   # idx stream position (in idxs)
            c0 = 0   # output column position
            call = 0
            from_masks = _build_masks(program)
            _, _, bm_off, key_idx = from_masks
            NBT = 7
            for (s, d, n_tiles, m) in program:
                S = d + 1
                ki = key_idx[(d, m)]
                t = 0
                while t < n_tiles:
                    nb = min(NBT, n_tiles - t)
                    nidx = nb * P
                    G = gp.tile([P, NBT, ROW], mybir.dt.bfloat16, tag="G")
                    nc.gpsimd.dma_gather(
                        out_ap=G[:, :nb, :],
                        in_ap=XP[int(seg_base[s]):int(seg_base[s + 1]), :],
                        idxs_ap=jtall[:, i0 // 16:(i0 + nidx) // 16],
                        num_idxs=nidx,
                        num_idxs_reg=nidx,
                        elem_size=ROW,
                        single_packet=False,
                        queue_num=call % NQUEUES,
                    )
                    call += 1
                    # ---- scores: one interleaved mul + reduce ----
                    # tmp[p, k, w, e] = x[p,k,e] * w2[w,e]  (w: 0=j, 1=i)
                    tmp = scp.tile([P, NBT, 2, HIDDEN], mybir.dt.bfloat16,
                                   tag="tmp")
                    nc.vector.tensor_mul(
                        tmp[:, :nb, :, :],
                        G[:, :nb, 1:HIDDEN + 1].unsqueeze(2).broadcast_to(
                            [P, nb, 2, HIDDEN]),
                        w2r[:].rearrange("p (w e) -> p w e", w=2)
                        .unsqueeze(1).broadcast_to([P, nb, 2, HIDDEN]))
                    sco = scp.tile([P, NBT, 2], mybir.dt.float32, tag="sco")
                    nc.vector.tensor_reduce(
                        sco[:, :nb, :], tmp[:, :nb, :, :],
                        axis=mybir.AxisListType.X, op=mybir.AluOpType.add)
                    Vc = scp.tile([P, NBT], mybir.dt.bfloat16, tag="Vc")
                    nc.vector.tensor_copy(Vc[:, :nb], sco[:, :nb, 1])
                    sib = psS.tile([P, NBT], mybir.dt.float32, tag="sib")
                    nc.tensor.matmul(
                        sib[:, :nb],
                        lhsT=small[:, ki * P:(ki + 1) * P],
                        rhs=Vc[:, :nb],
                        start=True, stop=True)
                    eraw = scp.tile([P, NBT], mybir.dt.float32, tag="eraw")
                    nc.vector.tensor_add(eraw[:, :nb], sib[:, :nb],
                                         sco[:, :nb, 0])
                    esc = scp.tile([P, NBT], mybir.dt.float32, tag="esc")
                    nc.vector.tensor_scalar_mul(esc[:, :nb], eraw[:, :nb],
                                                LEAKY)
                    elr = scp.tile([P, NBT], mybir.dt.float32, tag="elr")
                    nc.vector.tensor_max(elr[:, :nb], eraw[:, :nb],
                                         esc[:, :nb])
                    elc = scp.tile([P, NBT], mybir.dt.float32, tag="elc")
                    nc.vector.tensor_scalar_min(elc[:, :nb], elr[:, :nb],
                                                CLAMP)
                    ex = scp.tile([P, NBT], mybir.dt.float32, tag="ex")
                    nc.scalar.activation(ex[:, :nb], elc[:, :nb], AFT.Exp)
                    exsel = scp.tile([P, NBT, m], mybir.dt.bfloat16,
                                     tag="exsel")
                    nc.vector.tensor_mul(
                        exsel[:, :nb, :],
                        bmall[:, int(bm_off[ki]):int(bm_off[ki]) + m
                              ].unsqueeze(1).broadcast_to([P, nb, m]),
                        ex[:, :nb].unsqueeze(2).broadcast_to([P, nb, m]))
                    # ---- aggregation: per tile [m, 65] at free offset
                    # k*65 in one PSUM tile, then one relu/den/mul/store ----
                    U2 = psU.tile([32, NBT * (HIDDEN + 1)], mybir.dt.float32,
                                  tag="U2")
                    for k in range(nb):
                        nc.tensor.matmul(
                            U2[:m, k * (HIDDEN + 1):(k + 1) * (HIDDEN + 1)],
                            lhsT=exsel[:, k, :],
                            rhs=G[:, k, 0:HIDDEN + 1],
                            start=True, stop=True)
                    U3 = U2[:m, :nb * (HIDDEN + 1)].rearrange(
                        "p (k e) -> p k e", e=HIDDEN + 1)
                    Ur = flp.tile([32, NBT, HIDDEN + 1], mybir.dt.float32,
                                  tag="Ur")
                    nc.scalar.activation(
                        Ur[:m, :nb, :],
                        U3, AFT.Relu)
                    den = flp.tile([32, NBT], mybir.dt.float32, tag="den")
                    nc.vector.tensor_scalar_max(
                        den[:m, :nb], Ur[:m, :nb, 0], 1e-30)
                    rec = flp.tile([32, NBT], mybir.dt.float32, tag="rec")
                    nc.vector.reciprocal(rec[:m, :nb], den[:m, :nb])
                    ot = flp.tile([32, NBT, HIDDEN + 1], mybir.dt.float32,
                                  tag="ot")
                    nc.vector.tensor_mul(
                        ot[:m, :nb, :], Ur[:m, :nb, :],
                        rec[:m, :nb].unsqueeze(2).broadcast_to(
                            [m, nb, HIDDEN + 1]))
                    nc.sync.dma_start(
                        OUT[c0:c0 + m * nb, :],
                        ot[:m, :nb, :])
                    c0 += m * nb
                    i0 += nidx
                    t += nb
    nc.compile()
    return nc


def _install_profhook():
    """Register the axon NTFF profile hook (missing glue in this container)."""
    import contextlib
    import ctypes
    import sys
    import types

    if "antenv.axon_hooks" in sys.modules:
        return
    try:
        lib = ctypes.CDLL("/opt/axon/libaxon_pjrt.so")
        assert hasattr(lib, "axon_start_nrt_profile")
    except Exception:
        return
    lib.axon_start_nrt_profile.argtypes = [ctypes.POINTER(ctypes.c_int64),
                                           ctypes.c_size_t]
    lib.axon_start_nrt_profile.restype = ctypes.c_int64
    lib.axon_stop_nrt_profile.argtypes = [ctypes.c_char_p]
    lib.axon_stop_nrt_profile.restype = ctypes.c_int64

    @contextlib.contextmanager
    def _hook(output_dir, device_ids):
        import jax

        jax.devices()
        if device_ids:
            ids = (ctypes.c_int64 * len(device_ids))(*device_ids)
            rc = lib.axon_start_nrt_profile(ids, len(device_ids))
        else:
            rc = lib.axon_start_nrt_profile(None, 0)
        if rc != 0:
            raise RuntimeError(f"axon_start_nrt_profile rc={rc}")
        try:
            yield
        finally:
            lib.axon_stop_nrt_profile(str(output_dir).encode())

    mod = types.ModuleType("antenv.axon_hooks")
    mod.get_axon_ntff_profile_hook = lambda: _hook
    mod.set_axon_ntff_profile_hook = lambda h: None
    sys.modules["antenv.axon_hooks"] = mod
    import antenv

    antenv.axon_hooks = mod


def kernel(x, edge_index, w_i, w_j):
    import os
    import ml_dtypes
    from concourse.bass_utils import run_bass_kernel_spmd

    x = np.asarray(x, dtype=np.float32)
    edge_index = np.asarray(edge_index)
    w_i = np.asarray(w_i, dtype=np.float32)
    w_j = np.asarray(w_j, dtype=np.float32)
    n = x.shape[0]
    assert n == N_NODES and x.shape[1] == HIDDEN
    npc = n // N_CORES

    ej = edge_index[0].astype(np.int64)
    ei = edge_index[1].astype(np.int64)
    core_of = ei // npc
    edge_src, edge_dstl = [], []
    for c in range(N_CORES):
        sel = core_of == c
        edge_src.append(ej[sel])
        edge_dstl.append(ei[sel] - c * npc)

    program, total_tiles, total_cols, Js, colmaps, seg_nodes = \
        _build_layout(edge_src, edge_dstl, npc)
    bmc, smc, bm_off, key_idx = _build_masks(program)

    # per-segment table sizes must be uniform across cores for one program:
    # pad to the max across cores per segment
    seg_rows = [max(len(seg_nodes[c][s]) for c in range(N_CORES))
                for s in range(NSEG)]

    W2r = np.tile(np.concatenate([w_j, w_i]).astype(
        ml_dtypes.bfloat16)[None, :], (P, 1))

    nc = _build_program(program, total_tiles, total_cols, seg_rows,
                        bmc.shape[1], len(key_idx))

    in_maps = []
    for c in range(N_CORES):
        # pad each segment's table to the uniform size
        import ml_dtypes as mld
        tabs = []
        for s in range(NSEG):
            nodes = seg_nodes[c][s]
            t = np.zeros((seg_rows[s], ROW), dtype=mld.bfloat16)
            t[:len(nodes), 0] = 1.0
            t[:len(nodes), 1:HIDDEN + 1] = x[nodes].astype(mld.bfloat16)
            tabs.append(t)
        XPc = np.ascontiguousarray(np.concatenate(tabs, 0))
        idxmat = np.ascontiguousarray(Js[c].reshape(-1, 16).T)
        IDXc = np.ascontiguousarray(np.tile(idxmat, (8, 1)))
        in_maps.append({
            "XP": XPc, "IDX": IDXc,
            "W2R": np.ascontiguousarray(W2r),
            "BM": np.ascontiguousarray(bmc),
            "SM": np.ascontiguousarray(smc),
        })
    trace = os.environ.get("GAT_TRACE") == "1"
    if trace:
        _install_profhook()
    res = run_bass_kernel_spmd(nc, in_maps, core_ids=list(range(N_CORES)),
                               trace=trace)
    if trace and res.exec_time_ns:
        print(f"HW exec time: {res.exec_time_ns} ns")

    out = np.zeros((n, HIDDEN), dtype=np.float32)
    for c in range(N_CORES):
        ot = res.results[c]["OUT"][:, 1:]
        cm = colmaps[c]
        valid = cm >= 0
        out[c * npc + cm[valid]] = ot[valid, :]
    return out

